# revision 47
# baseline (speedup 1.0000x reference)
"""Trainium2 Bass kernel for nn_F2VConv3d (gnn message passing F2V conv).

v3: degree-sorted slot-aligned layout -- the scatter becomes PSUM
accumulation inside the depthwise GEMM.

  - Host: sort vertices by incident-edge count (desc), deal chunks of 1024
    round-robin to 8 cores x 98 blocks of 128 slots.  Block j needs
    T[j] = max degree in chunk j tiles; vertex at slot s stores its t-th
    edge at tile t, lane s.  Identical T profile on every core (SPMD).
    Host gathers per-edge input rows TRANSPOSED (inpT [128c, R]) and filt
    rows transposed+scaled by 1/max(nf_count,1), both bf16; zero filt
    columns kill padding lanes.  Filt is packed per BATCH (band i%3 at
    partitions 0/32/64, col block i//3) so the device can stream it in
    column chunks and start compute after the first chunk lands.
  - Device per batch of NT tiles (bf16 matmuls, f32 PSUM), software-
    pipelined with PE running PSW_BUFS-1 batches ahead:
      wT   = sw2_h.T @ filt-cols      (PE, 2 matmuls)
      featT= wT * inpT                (DVE direct from PSUM, or ACT-
                                       evacuate + Pool, alternating)
      pre += dw_h.T @ featT           (PE, accumulated in the block's
                                       [o,128slot] PSUM -- the segment-sum)
  - Per block: relu(pre+bias) -> bf16 stash
  - Quarter chunks: Copy+accum (sum) and Square+accum (sumsq) on ACT as
    relu data becomes available; [128,2] BN-stats AllReduce; fused affine
    (ACT Identity with per-partition scale+bias) + stores in 4 chunks.
BN statistics divide by true NV; padding slots give relu(0)=0, harmless.
"""
import numpy as np

NF, NV = 200000, 100000
C, M, K, CO = 128, 2, 16, 128
P = 128
NCORES = 8
BN_EPS = 1e-3
CHUNK = NCORES * P          # 1024 vertices per global chunk
B = (NV + CHUNK - 1) // CHUNK   # 98 blocks per core
NT_BATCH = 2                # tiles per wT-PSUM batch
PSW_BUFS = 6                # wT PSUM buffers; PE runs PSW_BUFS-1 ahead
PSO_BUFS = 2                # block-output PSUM buffers (relu slack)
POOL_MOD = 3                # batches with ctr%POOL_MOD in POOL_SET take the
POOL_SET = (2,)             # ACT-evacuate + Pool-multiply path
SLAB0_TILES = 24            # small first inpT slab (fast startup)
SLAB_TILES = 72             # steady-state inpT DMA slab size (tiles)
KBANDS = 3                  # filt k-bands at partitions 0/32/64
NFCHUNK = 4                 # filt DMA column chunks (first one smaller)
NQ = 8                      # stats chunks (smaller ACT ops, smaller bubbles)
NQA = 4                     # affine/store chunks
BLACKOUT = 5                # batches after a stats emission with no Pool path


def _batches(T):
    """Global batch list: (block j, tile offset t0, ntiles, first, last)."""
    items = []
    for j in range(B):
        Tj = int(T[j])
        t0 = 0
        blk = []
        while t0 < Tj:
            nt = min(NT_BATCH, Tj - t0)
            blk.append([j, t0, nt, False, False])
            t0 += nt
        blk[0][3] = True
        blk[-1][4] = True
        items.extend(tuple(x) for x in blk)
    return items


# ----------------------------------------------------------------------------
# host-side preprocessing
# ----------------------------------------------------------------------------

def _host_prep(face, vt_map, nf_count, filt_coeff, inputs):
    import ml_dtypes
    bf16 = ml_dtypes.bfloat16

    tgt = np.asarray(vt_map)[np.asarray(face)].ravel().astype(np.int64)
    nedge = tgt.shape[0]                       # 3*NF
    deg = np.bincount(tgt, minlength=NV)

    order = np.argsort(-deg, kind="stable")    # vertices by degree desc
    # vertex order[j*CHUNK + i] -> core i%8, block j, slot i//8
    vcore = np.empty(NV, dtype=np.int64)
    vblock = np.empty(NV, dtype=np.int64)
    vslot = np.empty(NV, dtype=np.int64)
    idx_in_chunk = np.arange(NV) % CHUNK
    chunk_id = np.arange(NV) // CHUNK
    vcore[order] = idx_in_chunk % NCORES
    vblock[order] = chunk_id
    vslot[order] = idx_in_chunk // NCORES

    # per-block tile count (same for all cores): max degree in chunk
    T = np.ones(B, dtype=np.int64)
    for j in range(B):
        T[j] = max(int(deg[order[j * CHUNK]]), 1)
    O = np.concatenate([[0], np.cumsum(T)])    # block tile offsets
    G = int(O[-1])                             # tiles per core
    R = G * P                                  # gather columns per core

    # occurrence index of each edge within its target vertex
    eord = np.argsort(tgt, kind="stable")
    counts = np.bincount(tgt, minlength=NV)
    starts = np.concatenate([[0], np.cumsum(counts)])
    occ = np.empty(nedge, dtype=np.int64)
    occ[eord] = np.arange(nedge) - starts[tgt[eord]]

    ecore = vcore[tgt]
    ecol = (O[vblock[tgt]] + occ) * P + vslot[tgt]   # column in core gather
    efid = np.arange(nedge, dtype=np.int64) // 3

    recip = (1.0 / np.maximum(np.asarray(nf_count), 1)).astype(np.float32)

    inp = np.asarray(inputs, dtype=np.float32)
    fc = np.asarray(filt_coeff, dtype=np.float32)

    items = _batches(T)
    ncb = (len(items) + KBANDS - 1) // KBANDS
    GBC = ncb * NT_BATCH * P                   # filt pack columns
    FPP = 32 * (KBANDS - 1) + 16               # filt pack partitions (80)

    inpT_cores, filtT_cores = [], []
    for c0 in range(NCORES):
        sel = ecore == c0
        cols = ecol[sel]
        fids = efid[sel]
        fidc = np.zeros(R, dtype=np.int64)
        scl = np.zeros(R, dtype=np.float32)
        fidc[cols] = fids
        scl[cols] = recip[tgt[sel]]
        inpT = np.ascontiguousarray(inp[fidc].astype(bf16).T)       # [128, R]
        filtT = (fc[fidc] * scl[:, None]).astype(bf16).T            # [16, R]
        fpack = np.zeros((FPP, GBC), dtype=bf16)
        for i, (j, t0, nt, _, _) in enumerate(items):
            a, cb = i % KBANDS, i // KBANDS
            g0 = int(O[j]) + t0
            fpack[32 * a:32 * a + 16,
                  cb * NT_BATCH * P:cb * NT_BATCH * P + nt * P] = \
                filtT[:, g0 * P:(g0 + nt) * P]
        inpT_cores.append(inpT)
        filtT_cores.append(np.ascontiguousarray(fpack))

    outcol = vblock * P + vslot
    return inpT_cores, filtT_cores, vcore, outcol, T, O, G


def _make_slabs(T, O):
    """Group blocks into DMA slabs: one small starter, then big ones."""
    slabs = []
    cur = []
    lo = 0
    limit = SLAB0_TILES
    for j in range(B):
        cur.append(j)
        if O[j + 1] - lo >= limit or j == B - 1:
            slabs.append((int(lo), int(O[j + 1]), list(cur)))
            lo = int(O[j + 1])
            cur = []
            limit = SLAB_TILES
    return slabs


# ----------------------------------------------------------------------------
# device kernel
# ----------------------------------------------------------------------------

def _build_kernel(T, O, G, with_collective=True):
    import concourse.bass as bass
    import concourse.bacc as bacc
    import concourse.mybir as mybir
    import concourse.tile as tile

    f32 = mybir.dt.float32
    bf16 = mybir.dt.bfloat16
    AF = mybir.ActivationFunctionType
    ALU = mybir.AluOpType

    R = G * P
    W = B * P
    slabs = _make_slabs(T, O)
    items = _batches(T)
    nb = len(items)
    ncb = (nb + KBANDS - 1) // KBANDS
    GBC = ncb * NT_BATCH * P
    FPP = 32 * (KBANDS - 1) + 16

    # filt DMA chunk boundaries in cb space; small first chunk so the
    # first wT can start early
    first_cb = max(1, ncb // 8)
    rest = [first_cb + round(c * (ncb - first_cb) / (NFCHUNK - 1))
            for c in range(NFCHUNK)]
    fchunk_cb = [0] + rest
    fchunk_cb[-1] = ncb

    nc = bacc.Bacc()
    inpT_d = nc.dram_tensor("inpT", [P, R], bf16, kind="ExternalInput")
    filtT_d = nc.dram_tensor("filtT", [FPP, GBC], bf16, kind="ExternalInput")
    sw2_d = nc.dram_tensor("sw2", [FPP, M * C], bf16, kind="ExternalInput")
    dw2_d = nc.dram_tensor("dw2", [M * C, CO], bf16, kind="ExternalInput")
    cpack_d = nc.dram_tensor("constpack", [P, 3], f32, kind="ExternalInput")
    out_d = nc.dram_tensor("out_t", [P, W], bf16, kind="ExternalOutput")

    with tile.TileContext(nc) as tc:
        with (
            tc.tile_pool(name="const", bufs=1) as cpool,
            tc.tile_pool(name="slab", bufs=3) as spool,
            tc.tile_pool(name="big", bufs=1) as bigpool,
            tc.tile_pool(name="feat", bufs=6) as fpool,
            tc.tile_pool(name="wsb", bufs=3) as wpool_sb,
            tc.tile_pool(name="small", bufs=2) as smpool,
            tc.tile_pool(name="ps_w", bufs=PSW_BUFS, space="PSUM") as ps_w,
            tc.tile_pool(name="ps_o", bufs=PSO_BUFS, space="PSUM") as ps_o,
            tc.tile_pool(name="dram", bufs=1, space="DRAM") as dpool,
        ):
            # ---- constants; order matters for fast startup: the first wT
            # needs sw2 + filt chunk 0, the first mult needs inp slab 0
            # (issued below on the SP ring), dw/cpk only at GEMM/relu time.
            sw2 = cpool.tile([FPP, M * C], bf16)
            nc.scalar.dma_start(out=sw2[:], in_=sw2_d[:])
            fchunks = []
            fch_tiles = []
            for ci in range(NFCHUNK):
                clo, chi = fchunk_cb[ci], fchunk_cb[ci + 1]
                ft = cpool.tile([FPP, (chi - clo) * NT_BATCH * P], bf16)
                fch_tiles.append(ft)
                fchunks.append(ft)
            nc.scalar.dma_start(
                out=fch_tiles[0][:],
                in_=filtT_d[:, fchunk_cb[0] * NT_BATCH * P:
                            fchunk_cb[1] * NT_BATCH * P])
            dw_a = cpool.tile([P, CO], bf16)
            dw_b = cpool.tile([P, CO], bf16)
            nc.scalar.dma_start(out=dw_a[:], in_=dw2_d[0:P, :])
            nc.scalar.dma_start(out=dw_b[:], in_=dw2_d[P:2 * P, :])
            cpk = cpool.tile([P, 3], f32)
            nc.scalar.dma_start(out=cpk[:], in_=cpack_d[:])
            for ci in range(1, NFCHUNK):
                clo, chi = fchunk_cb[ci], fchunk_cb[ci + 1]
                nc.scalar.dma_start(
                    out=fch_tiles[ci][:],
                    in_=filtT_d[:, clo * NT_BATCH * P:chi * NT_BATCH * P])
            bias_c = cpk[:, 0:1]
            gamma_c = cpk[:, 1:2]
            beta_c = cpk[:, 2:3]

            relu_buf = bigpool.tile([P, W], bf16, tag="relu_buf")
            out_bf = bigpool.tile([P, W], bf16, tag="out_bf")
            sscr = bigpool.tile([P, (B // NQ + 1) * P], bf16, tag="sscr")
            s_parts = bigpool.tile([P, NQ], f32, tag="s_parts")
            ss_parts = bigpool.tile([P, NQ], f32, tag="ss_parts")

            # stats chunk boundaries in block space
            qb = [round(q * B / NQ) for q in range(NQ + 1)]
            # affine/store chunk boundaries
            qa = [round(q * B / NQA) for q in range(NQA + 1)]

            block_slab = {}
            for si, (lo, hi, bl) in enumerate(slabs):
                for j in bl:
                    block_slab[j] = si
            slab_tiles = {}

            def load_slab(si):
                if si < len(slabs) and si not in slab_tiles:
                    lo, hi, _ = slabs[si]
                    st = spool.tile([P, (hi - lo) * P], bf16, tag="inp_slab")
                    nc.sync.dma_start(out=st[:], in_=inpT_d[:, lo * P:hi * P])
                    slab_tiles[si] = (st, lo)

            def ensure_slab(j):
                si = block_slab[j]
                load_slab(si)
                load_slab(si + 1)   # chain-prefetch a full slab ahead

            def do_wT(i):
                j, t0, nt, _, _ = items[i]
                a, cb = i % KBANDS, i // KBANDS
                ci = next(c for c in range(NFCHUNK)
                          if fchunk_cb[c] <= cb < fchunk_cb[c + 1])
                cb_loc = cb - fchunk_cb[ci]
                fcol = cb_loc * NT_BATCH * P
                w_ps = ps_w.tile([P, nt * M * C], f32, tag="w")
                for h in range(2):
                    wsl = w_ps[:, h * P:h * P + P]
                    if nt > 1:
                        wout = bass.AP(wsl.tensor, wsl.offset,
                                       [wsl.ap[0], [M * C, nt], [1, P]])
                    else:
                        wout = wsl
                    nc.tensor.matmul(
                        out=wout,
                        lhsT=sw2[32 * a:32 * a + 16, h * P:h * P + P],
                        rhs=fchunks[ci][32 * a:32 * a + 16,
                                        fcol:fcol + nt * P],
                        start=True, stop=True,
                    )
                return w_ps

            # ---- main pass (PE runs PSW_BUFS-1 batches ahead)
            RUNAHEAD = PSW_BUFS - 1
            outp = None
            w_q = []
            qi = 0
            blackout = 0
            for idx, item in enumerate(items):
                j, t0, nt, first, last = item
                ensure_slab(j)
                slab_t, slab_lo = slab_tiles[block_slab[j]]
                base_t = int(O[j]) - slab_lo
                if idx == 0:
                    for k in range(min(RUNAHEAD, nb)):
                        ensure_slab(items[k][0])
                        w_q.append(do_wT(k))
                if idx + RUNAHEAD < nb:
                    ensure_slab(items[idx + RUNAHEAD][0])
                    w_q.append(do_wT(idx + RUNAHEAD))
                w_ps = w_q.pop(0)
                if first:
                    outp = ps_o.tile([P, P], f32, tag="outp")

                featT = fpool.tile([P, nt * M * C], bf16, tag="featT")
                fcol = (base_t + t0) * P
                isl = slab_t[:, fcol:fcol + P]
                in1 = bass.AP(isl.tensor, isl.offset,
                              [isl.ap[0], [P, nt], [0, 2], [1, P]])
                if (idx % POOL_MOD) in POOL_SET and blackout == 0:
                    # Pool cannot read PSUM: ACT evacuates wT as bf16.
                    wsb = wpool_sb.tile([P, nt * M * C], bf16, tag="wsb")
                    nc.scalar.copy(out=wsb[:], in_=w_ps[:])
                    nc.gpsimd.tensor_tensor(out=featT[:], in0=wsb[:],
                                            in1=in1, op=ALU.mult)
                else:
                    nc.vector.tensor_tensor(out=featT[:], in0=w_ps[:],
                                            in1=in1, op=ALU.mult)
                if blackout > 0:
                    blackout -= 1
                # GEMM, batched per h with a stride-0 accumulating out AP.
                # The first instruction of the block covers each psum
                # address exactly once (start=True resets).
                for h in range(2):
                    fob = (first and h == 0)
                    tlo = 0
                    if fob:
                        nc.tensor.matmul(
                            out=outp[:], lhsT=dw_a[:],
                            rhs=featT[:, h * P:h * P + P],
                            start=True, stop=False)
                        tlo = 1
                        if nt == 1:
                            continue
                    ntb = nt - tlo
                    fsl = featT[:, (2 * tlo + h) * P:(2 * tlo + h) * P + P]
                    if ntb > 1:
                        rhs = bass.AP(fsl.tensor, fsl.offset,
                                      [fsl.ap[0], [M * C, ntb], [1, P]])
                        oap = bass.AP(outp[:].tensor, outp[:].offset,
                                      [outp[:].ap[0], [0, ntb], [1, P]])
                    else:
                        rhs = fsl
                        oap = outp[:]
                    nc.tensor.matmul(
                        out=oap, lhsT=(dw_a if h == 0 else dw_b)[:],
                        rhs=rhs, start=False, stop=(last and h == 1))

                if last:
                    nc.scalar.activation(out=relu_buf[:, j * P:(j + 1) * P],
                                         in_=outp[:], func=AF.Relu,
                                         bias=bias_c)
                    # chunked BN statistics as soon as data is ready
                    if j + 1 == qb[qi + 1]:
                        lo, hi = qb[qi] * P, qb[qi + 1] * P
                        nc.scalar.activation(
                            out=sscr[:, 0:hi - lo], in_=relu_buf[:, lo:hi],
                            func=AF.Square,
                            accum_out=ss_parts[:, qi:qi + 1])
                        nc.scalar.activation(
                            out=sscr[:, 0:hi - lo], in_=relu_buf[:, lo:hi],
                            func=AF.Copy,
                            accum_out=s_parts[:, qi:qi + 1])
                        qi += 1
                        blackout = BLACKOUT

            # ---- BN statistics
            stats = smpool.tile([P, 2], f32, tag="stats")
            nc.vector.reduce_sum(out=stats[:, 0:1], in_=s_parts[:],
                                 axis=mybir.AxisListType.X)
            nc.vector.reduce_sum(out=stats[:, 1:2], in_=ss_parts[:],
                                 axis=mybir.AxisListType.X)

            cc_in = dpool.tile([P, 2], f32, tag="cc_in")
            cc_out = dpool.tile([P, 2], f32, tag="cc_out")
            nc.gpsimd.dma_start(out=cc_in[:], in_=stats[:])
            if with_collective:
                nc.gpsimd.collective_compute(
                    "AllReduce", ALU.add,
                    replica_groups=[list(range(NCORES))],
                    ins=[cc_in.opt()], outs=[cc_out.opt()],
                )
            else:
                nc.gpsimd.dma_start(out=cc_out[:], in_=cc_in[:])
            stats_g = smpool.tile([P, 2], f32, tag="stats_g")
            nc.gpsimd.dma_start(out=stats_g[:], in_=cc_out[:])

            mean = smpool.tile([P, 1], f32, tag="mean")
            nc.vector.tensor_scalar(out=mean[:], in0=stats_g[:, 0:1],
                                    scalar1=1.0 / NV, scalar2=None, op0=ALU.mult)
            ex2 = smpool.tile([P, 1], f32, tag="ex2")
            nc.vector.tensor_scalar(out=ex2[:], in0=stats_g[:, 1:2],
                                    scalar1=1.0 / NV, scalar2=None, op0=ALU.mult)
            msq = smpool.tile([P, 1], f32, tag="msq")
            nc.vector.tensor_tensor(out=msq[:], in0=mean[:], in1=mean[:],
                                    op=ALU.mult)
            var = smpool.tile([P, 1], f32, tag="var")
            nc.vector.tensor_tensor(out=var[:], in0=ex2[:], in1=msq[:],
                                    op=ALU.subtract)
            vare = smpool.tile([P, 1], f32, tag="vare")
            nc.vector.tensor_scalar(out=vare[:], in0=var[:], scalar1=BN_EPS,
                                    scalar2=None, op0=ALU.add)
            std = smpool.tile([P, 1], f32, tag="std")
            nc.scalar.activation(out=std[:], in_=vare[:], func=AF.Sqrt)
            rstd = smpool.tile([P, 1], f32, tag="rstd")
            nc.vector.reciprocal(out=rstd[:], in_=std[:])
            scale = smpool.tile([P, 1], f32, tag="scale")
            nc.vector.tensor_tensor(out=scale[:], in0=gamma_c, in1=rstd[:],
                                    op=ALU.mult)
            nshift = smpool.tile([P, 1], f32, tag="nshift")
            nc.vector.tensor_tensor(out=nshift[:], in0=mean[:], in1=scale[:],
                                    op=ALU.mult)
            shift = smpool.tile([P, 1], f32, tag="shift")
            nc.vector.tensor_tensor(out=shift[:], in0=beta_c, in1=nshift[:],
                                    op=ALU.subtract)

            # ---- fused BN affine + stores, chunked, spread across ACT /
            # DVE / Pool so the chunks run in parallel; stores alternate
            # between the two HWDGE rings.
            aff_eng = [None, nc.vector, None, nc.vector]
            for q in range(NQA):
                lo, hi = qa[q] * P, qa[q + 1] * P
                eng = aff_eng[q % len(aff_eng)]
                if eng is None:
                    nc.scalar.activation(out=out_bf[:, lo:hi],
                                         in_=relu_buf[:, lo:hi],
                                         func=AF.Identity,
                                         scale=scale[:, 0:1],
                                         bias=shift[:, 0:1])
                else:
                    eng.scalar_tensor_tensor(
                        out=out_bf[:, lo:hi], in0=relu_buf[:, lo:hi],
                        scalar=scale[:, 0:1],
                        in1=shift[:, 0:1].to_broadcast([P, hi - lo]),
                        op0=ALU.mult, op1=ALU.add)
                ring = nc.sync if q % 2 == 0 else nc.scalar
                ring.dma_start(out=out_d[:, lo:hi], in_=out_bf[:, lo:hi])

    nc.finalize()
    return nc


# ----------------------------------------------------------------------------
# entry point
# ----------------------------------------------------------------------------

def prepare(inp_dict):
    """Host prep + build device program + per-core input maps (no execution)."""
    import ml_dtypes
    bf16 = ml_dtypes.bfloat16

    inputs = inp_dict["inputs"]
    filt_coeff = inp_dict["filt_coeff"]
    face = inp_dict["face"]
    nf_count = inp_dict["nf_count"]
    vt_map = inp_dict["vt_map"]
    spatial_weights = inp_dict["spatial_weights"]
    depth_weights = inp_dict["depth_weights"]
    biases = inp_dict["biases"]
    gamma = inp_dict["gamma"]
    beta = inp_dict["beta"]

    inpT_cores, filtT_cores, vcore, outcol, T, O, G = _host_prep(
        face, vt_map, nf_count, filt_coeff, inputs)

    # m-major (m,c) layouts so chunk h == m; sw2 replicated at partition
    # bases 0/32/64 to match the filt k-band the PE reads.
    sw2_16 = np.ascontiguousarray(
        np.asarray(spatial_weights, dtype=np.float32)
        .transpose(0, 2, 1).reshape(16, M * C)).astype(bf16)
    FPP = 32 * (KBANDS - 1) + 16
    sw2 = np.zeros((FPP, M * C), dtype=bf16)
    for a in range(KBANDS):
        sw2[32 * a:32 * a + 16] = sw2_16
    dw2 = np.ascontiguousarray(
        np.asarray(depth_weights, dtype=np.float32).reshape(C, M, CO)
        .transpose(1, 0, 2).reshape(M * C, CO)).astype(bf16)

    cp = np.zeros((P, 3), dtype=np.float32)
    cp[:, 0] = np.asarray(biases, dtype=np.float32).reshape(CO)
    cp[:, 1] = np.asarray(gamma, dtype=np.float32).reshape(CO)
    cp[:, 2] = np.asarray(beta, dtype=np.float32).reshape(CO)

    nc = _build_kernel(T, O, G)

    in_maps = []
    for c0 in range(NCORES):
        in_maps.append({
            "inpT": inpT_cores[c0],
            "filtT": filtT_cores[c0],
            "sw2": sw2,
            "dw2": dw2,
            "constpack": cp,
        })

    return {"nc": nc, "in_maps": in_maps, "ncores": NCORES,
            "vcore": vcore, "outcol": outcol, "T": T, "O": O, "G": G}


def kernel(inputs, filt_coeff, face, nf_count, vt_map,
           spatial_weights, depth_weights, biases, gamma, beta):
    from concourse.bass_utils import run_bass_kernel_spmd

    prep = prepare(dict(
        inputs=inputs, filt_coeff=filt_coeff, face=face, nf_count=nf_count,
        vt_map=vt_map, spatial_weights=spatial_weights,
        depth_weights=depth_weights, biases=biases, gamma=gamma, beta=beta))
    nc, in_maps = prep["nc"], prep["in_maps"]
    vcore, outcol = prep["vcore"], prep["outcol"]

    res = run_bass_kernel_spmd(nc, in_maps, core_ids=list(range(NCORES)))
    global _last_results
    _last_results = res
    out = np.zeros((NV, CO), dtype=np.float32)
    percore = [np.asarray(res.results[c]["out_t"], dtype=np.float32)
               for c in range(NCORES)]    # [128o, B*128]
    for c0 in range(NCORES):
        vs = np.where(vcore == c0)[0]
        out[vs] = percore[c0][:, outcol[vs]].T
    return out


# revision 50
# speedup vs baseline: 1.6223x; 1.6223x over previous
"""Trainium2 Bass kernel for nn_F2VConv3d (gnn message passing F2V conv).

v3: degree-sorted slot-aligned layout -- the scatter becomes PSUM
accumulation inside the depthwise GEMM.

  - Host: sort vertices by incident-edge count (desc), deal chunks of 1024
    round-robin to 8 cores x 98 blocks of 128 slots.  Block j needs
    T[j] = max degree in chunk j tiles; vertex at slot s stores its t-th
    edge at tile t, lane s.  Identical T profile on every core (SPMD).
    Host gathers per-edge input rows TRANSPOSED (inpT [128c, R]) and filt
    rows transposed+scaled by 1/max(nf_count,1), both bf16; zero filt
    columns kill padding lanes.  Filt is packed per BATCH (band i%3 at
    partitions 0/32/64, col block i//3) so the device can stream it in
    column chunks and start compute after the first chunk lands.
  - Device per batch of NT tiles (bf16 matmuls, f32 PSUM), software-
    pipelined with PE running PSW_BUFS-1 batches ahead:
      wT   = sw2_h.T @ filt-cols      (PE, 2 matmuls)
      featT= wT * inpT                (DVE direct from PSUM, or ACT-
                                       evacuate + Pool, alternating)
      pre += dw_h.T @ featT           (PE, accumulated in the block's
                                       [o,128slot] PSUM -- the segment-sum)
  - Per block: relu(pre+bias) -> bf16 stash
  - Quarter chunks: Copy+accum (sum) and Square+accum (sumsq) on ACT as
    relu data becomes available; [128,2] BN-stats AllReduce; fused affine
    (ACT Identity with per-partition scale+bias) + stores in 4 chunks.
BN statistics divide by true NV; padding slots give relu(0)=0, harmless.
"""
import numpy as np

NF, NV = 200000, 100000
C, M, K, CO = 128, 2, 16, 128
P = 128
NCORES = 8
BN_EPS = 1e-3
CHUNK = NCORES * P          # 1024 vertices per global chunk
B = (NV + CHUNK - 1) // CHUNK   # 98 blocks per core
NT_BATCH = 2                # tiles per wT-PSUM batch
PSW_BUFS = 6                # wT PSUM buffers; PE runs PSW_BUFS-1 ahead
PSO_BUFS = 2                # block-output PSUM buffers (relu slack)
POOL_MOD = 3                # batches with ctr%POOL_MOD in POOL_SET take the
POOL_SET = (2,)             # ACT-evacuate + Pool-multiply path
SLAB0_TILES = 24            # small first inpT slab (fast startup)
SLAB_TILES = 72             # steady-state inpT DMA slab size (tiles)
KBANDS = 3                  # filt k-bands at partitions 0/32/64
NFCHUNK = 4                 # filt DMA column chunks (first one smaller)
NQ = 8                      # stats chunks (smaller ACT ops, smaller bubbles)
NQA = 4                     # affine/store chunks
BLACKOUT = 5                # batches after a stats emission with no Pool path


def _batches(T):
    """Global batch list: (block j, tile offset t0, ntiles, first, last)."""
    items = []
    for j in range(B):
        Tj = int(T[j])
        t0 = 0
        blk = []
        while t0 < Tj:
            nt = min(NT_BATCH, Tj - t0)
            blk.append([j, t0, nt, False, False])
            t0 += nt
        blk[0][3] = True
        blk[-1][4] = True
        items.extend(tuple(x) for x in blk)
    return items


# ----------------------------------------------------------------------------
# host-side preprocessing
# ----------------------------------------------------------------------------

def _host_prep(face, vt_map, nf_count, filt_coeff, inputs):
    import ml_dtypes
    bf16 = ml_dtypes.bfloat16

    tgt = np.asarray(vt_map)[np.asarray(face)].ravel().astype(np.int64)
    nedge = tgt.shape[0]                       # 3*NF
    deg = np.bincount(tgt, minlength=NV)

    order = np.argsort(-deg, kind="stable")    # vertices by degree desc
    # vertex order[j*CHUNK + i] -> core i%8, block j, slot i//8
    vcore = np.empty(NV, dtype=np.int64)
    vblock = np.empty(NV, dtype=np.int64)
    vslot = np.empty(NV, dtype=np.int64)
    idx_in_chunk = np.arange(NV) % CHUNK
    chunk_id = np.arange(NV) // CHUNK
    vcore[order] = idx_in_chunk % NCORES
    vblock[order] = chunk_id
    vslot[order] = idx_in_chunk // NCORES

    # per-block tile count (same for all cores): max degree in chunk
    T = np.ones(B, dtype=np.int64)
    for j in range(B):
        T[j] = max(int(deg[order[j * CHUNK]]), 1)
    O = np.concatenate([[0], np.cumsum(T)])    # block tile offsets
    G = int(O[-1])                             # tiles per core
    R = G * P                                  # gather columns per core

    # occurrence index of each edge within its target vertex
    eord = np.argsort(tgt, kind="stable")
    counts = np.bincount(tgt, minlength=NV)
    starts = np.concatenate([[0], np.cumsum(counts)])
    occ = np.empty(nedge, dtype=np.int64)
    occ[eord] = np.arange(nedge) - starts[tgt[eord]]

    ecore = vcore[tgt]
    ecol = (O[vblock[tgt]] + occ) * P + vslot[tgt]   # column in core gather
    efid = np.arange(nedge, dtype=np.int64) // 3

    recip = (1.0 / np.maximum(np.asarray(nf_count), 1)).astype(np.float32)

    inp = np.asarray(inputs, dtype=np.float32)
    fc = np.asarray(filt_coeff, dtype=np.float32)

    items = _batches(T)
    ncb = (len(items) + KBANDS - 1) // KBANDS
    GBC = ncb * NT_BATCH * P                   # filt pack columns
    FPP = 32 * (KBANDS - 1) + 16               # filt pack partitions (80)

    inpT_cores, filtT_cores = [], []
    for c0 in range(NCORES):
        sel = ecore == c0
        cols = ecol[sel]
        fids = efid[sel]
        fidc = np.zeros(R, dtype=np.int64)
        scl = np.zeros(R, dtype=np.float32)
        fidc[cols] = fids
        scl[cols] = recip[tgt[sel]]
        inpT = np.ascontiguousarray(inp[fidc].astype(bf16).T)       # [128, R]
        filtT = (fc[fidc] * scl[:, None]).astype(bf16).T            # [16, R]
        fpack = np.zeros((FPP, GBC), dtype=bf16)
        for i, (j, t0, nt, _, _) in enumerate(items):
            a, cb = i % KBANDS, i // KBANDS
            g0 = int(O[j]) + t0
            fpack[32 * a:32 * a + 16,
                  cb * NT_BATCH * P:cb * NT_BATCH * P + nt * P] = \
                filtT[:, g0 * P:(g0 + nt) * P]
        inpT_cores.append(inpT)
        filtT_cores.append(np.ascontiguousarray(fpack))

    outcol = vblock * P + vslot
    return inpT_cores, filtT_cores, vcore, outcol, T, O, G


def _make_slabs(T, O):
    """Group blocks into DMA slabs: one small starter, then big ones."""
    slabs = []
    cur = []
    lo = 0
    limit = SLAB0_TILES
    for j in range(B):
        cur.append(j)
        if O[j + 1] - lo >= limit or j == B - 1:
            slabs.append((int(lo), int(O[j + 1]), list(cur)))
            lo = int(O[j + 1])
            cur = []
            limit = SLAB_TILES
    return slabs


# ----------------------------------------------------------------------------
# device kernel
# ----------------------------------------------------------------------------

def _build_kernel(T, O, G, with_collective=True):
    import concourse.bass as bass
    import concourse.bacc as bacc
    import concourse.mybir as mybir
    import concourse.tile as tile

    f32 = mybir.dt.float32
    bf16 = mybir.dt.bfloat16
    AF = mybir.ActivationFunctionType
    ALU = mybir.AluOpType

    R = G * P
    W = B * P
    slabs = _make_slabs(T, O)
    items = _batches(T)
    nb = len(items)
    ncb = (nb + KBANDS - 1) // KBANDS
    GBC = ncb * NT_BATCH * P
    FPP = 32 * (KBANDS - 1) + 16

    # filt DMA chunk boundaries in cb space; small first chunk so the
    # first wT can start early
    first_cb = max(1, ncb // 8)
    rest = [first_cb + round(c * (ncb - first_cb) / (NFCHUNK - 1))
            for c in range(NFCHUNK)]
    fchunk_cb = [0] + rest
    fchunk_cb[-1] = ncb

    nc = bacc.Bacc()
    inpT_d = nc.dram_tensor("inpT", [P, R], bf16, kind="ExternalInput")
    filtT_d = nc.dram_tensor("filtT", [FPP, GBC], bf16, kind="ExternalInput")
    sw2_d = nc.dram_tensor("sw2", [FPP, M * C], bf16, kind="ExternalInput")
    dw2_d = nc.dram_tensor("dw2", [M * C, CO], bf16, kind="ExternalInput")
    cpack_d = nc.dram_tensor("constpack", [P, 3], f32, kind="ExternalInput")
    out_d = nc.dram_tensor("out_t", [P, W], bf16, kind="ExternalOutput")

    with tile.TileContext(nc) as tc:
        with (
            tc.tile_pool(name="const", bufs=1) as cpool,
            tc.tile_pool(name="slab", bufs=3) as spool,
            tc.tile_pool(name="big", bufs=1) as bigpool,
            tc.tile_pool(name="feat", bufs=6) as fpool,
            tc.tile_pool(name="wsb", bufs=3) as wpool_sb,
            tc.tile_pool(name="small", bufs=2) as smpool,
            tc.tile_pool(name="ps_w", bufs=PSW_BUFS, space="PSUM") as ps_w,
            tc.tile_pool(name="ps_o", bufs=PSO_BUFS, space="PSUM") as ps_o,
            tc.tile_pool(name="dram", bufs=1, space="DRAM") as dpool,
        ):
            # ---- constants; order matters for fast startup: the first wT
            # needs sw2 + filt chunk 0, the first mult needs inp slab 0
            # (issued below on the SP ring), dw/cpk only at GEMM/relu time.
            sw2 = cpool.tile([FPP, M * C], bf16)
            nc.scalar.dma_start(out=sw2[:], in_=sw2_d[:])
            fchunks = []
            fch_tiles = []
            for ci in range(NFCHUNK):
                clo, chi = fchunk_cb[ci], fchunk_cb[ci + 1]
                ft = cpool.tile([FPP, (chi - clo) * NT_BATCH * P], bf16)
                fch_tiles.append(ft)
                fchunks.append(ft)
            nc.scalar.dma_start(
                out=fch_tiles[0][:],
                in_=filtT_d[:, fchunk_cb[0] * NT_BATCH * P:
                            fchunk_cb[1] * NT_BATCH * P])
            dw_a = cpool.tile([P, CO], bf16)
            dw_b = cpool.tile([P, CO], bf16)
            nc.scalar.dma_start(out=dw_a[:], in_=dw2_d[0:P, :])
            nc.scalar.dma_start(out=dw_b[:], in_=dw2_d[P:2 * P, :])
            cpk = cpool.tile([P, 3], f32)
            nc.scalar.dma_start(out=cpk[:], in_=cpack_d[:])
            bias_c = cpk[:, 0:1]
            gamma_c = cpk[:, 1:2]
            beta_c = cpk[:, 2:3]

            relu_buf = bigpool.tile([P, W], bf16, tag="relu_buf")
            out_bf = bigpool.tile([P, W], bf16, tag="out_bf")
            sscr = bigpool.tile([P, (B // NQ + 1) * P], bf16, tag="sscr")
            s_parts = bigpool.tile([P, NQ], f32, tag="s_parts")
            ss_parts = bigpool.tile([P, NQ], f32, tag="ss_parts")

            # stats chunk boundaries in block space
            qb = [round(q * B / NQ) for q in range(NQ + 1)]
            # affine/store chunk boundaries
            qa = [round(q * B / NQA) for q in range(NQA + 1)]

            block_slab = {}
            for si, (lo, hi, bl) in enumerate(slabs):
                for j in bl:
                    block_slab[j] = si
            slab_tiles = {}

            def load_slab(si):
                if si < len(slabs) and si not in slab_tiles:
                    lo, hi, _ = slabs[si]
                    st = spool.tile([P, (hi - lo) * P], bf16, tag="inp_slab")
                    nc.sync.dma_start(out=st[:], in_=inpT_d[:, lo * P:hi * P])
                    slab_tiles[si] = (st, lo)

            def ensure_slab(j):
                si = block_slab[j]
                load_slab(si)
                load_slab(si + 1)   # chain-prefetch a full slab ahead

            # first two inp slabs BEFORE the remaining filt chunks so the
            # first mults aren't queued behind ~9us of filt DMA traffic
            load_slab(0)
            load_slab(1)
            for ci in range(1, NFCHUNK):
                clo, chi = fchunk_cb[ci], fchunk_cb[ci + 1]
                nc.scalar.dma_start(
                    out=fch_tiles[ci][:],
                    in_=filtT_d[:, clo * NT_BATCH * P:chi * NT_BATCH * P])

            def do_wT(i):
                j, t0, nt, _, _ = items[i]
                a, cb = i % KBANDS, i // KBANDS
                ci = next(c for c in range(NFCHUNK)
                          if fchunk_cb[c] <= cb < fchunk_cb[c + 1])
                cb_loc = cb - fchunk_cb[ci]
                fcol = cb_loc * NT_BATCH * P
                w_ps = ps_w.tile([P, nt * M * C], f32, tag="w")
                for h in range(2):
                    wsl = w_ps[:, h * P:h * P + P]
                    if nt > 1:
                        wout = bass.AP(wsl.tensor, wsl.offset,
                                       [wsl.ap[0], [M * C, nt], [1, P]])
                    else:
                        wout = wsl
                    nc.tensor.matmul(
                        out=wout,
                        lhsT=sw2[32 * a:32 * a + 16, h * P:h * P + P],
                        rhs=fchunks[ci][32 * a:32 * a + 16,
                                        fcol:fcol + nt * P],
                        start=True, stop=True,
                    )
                return w_ps

            # ---- main pass (PE runs PSW_BUFS-1 batches ahead)
            RUNAHEAD = PSW_BUFS - 1
            outp = None
            w_q = []
            qi = 0
            blackout = 0
            for idx, item in enumerate(items):
                j, t0, nt, first, last = item
                ensure_slab(j)
                slab_t, slab_lo = slab_tiles[block_slab[j]]
                base_t = int(O[j]) - slab_lo
                if idx == 0:
                    for k in range(min(RUNAHEAD, nb)):
                        ensure_slab(items[k][0])
                        w_q.append(do_wT(k))
                if idx + RUNAHEAD < nb:
                    ensure_slab(items[idx + RUNAHEAD][0])
                    w_q.append(do_wT(idx + RUNAHEAD))
                w_ps = w_q.pop(0)
                if first:
                    outp = ps_o.tile([P, P], f32, tag="outp")

                featT = fpool.tile([P, nt * M * C], bf16, tag="featT")
                fcol = (base_t + t0) * P
                isl = slab_t[:, fcol:fcol + P]
                in1 = bass.AP(isl.tensor, isl.offset,
                              [isl.ap[0], [P, nt], [0, 2], [1, P]])
                if (idx % POOL_MOD) in POOL_SET and blackout == 0:
                    # Pool cannot read PSUM: ACT evacuates wT as bf16.
                    wsb = wpool_sb.tile([P, nt * M * C], bf16, tag="wsb")
                    nc.scalar.copy(out=wsb[:], in_=w_ps[:])
                    nc.gpsimd.tensor_tensor(out=featT[:], in0=wsb[:],
                                            in1=in1, op=ALU.mult)
                else:
                    nc.vector.tensor_tensor(out=featT[:], in0=w_ps[:],
                                            in1=in1, op=ALU.mult)
                if blackout > 0:
                    blackout -= 1
                # GEMM, batched per h with a stride-0 accumulating out AP.
                # The first instruction of the block covers each psum
                # address exactly once (start=True resets).
                for h in range(2):
                    fob = (first and h == 0)
                    tlo = 0
                    if fob:
                        nc.tensor.matmul(
                            out=outp[:], lhsT=dw_a[:],
                            rhs=featT[:, h * P:h * P + P],
                            start=True, stop=False)
                        tlo = 1
                        if nt == 1:
                            continue
                    ntb = nt - tlo
                    fsl = featT[:, (2 * tlo + h) * P:(2 * tlo + h) * P + P]
                    if ntb > 1:
                        rhs = bass.AP(fsl.tensor, fsl.offset,
                                      [fsl.ap[0], [M * C, ntb], [1, P]])
                        oap = bass.AP(outp[:].tensor, outp[:].offset,
                                      [outp[:].ap[0], [0, ntb], [1, P]])
                    else:
                        rhs = fsl
                        oap = outp[:]
                    nc.tensor.matmul(
                        out=oap, lhsT=(dw_a if h == 0 else dw_b)[:],
                        rhs=rhs, start=False, stop=(last and h == 1))

                if last:
                    nc.scalar.activation(out=relu_buf[:, j * P:(j + 1) * P],
                                         in_=outp[:], func=AF.Relu,
                                         bias=bias_c)
                    # chunked BN statistics as soon as data is ready
                    if j + 1 == qb[qi + 1]:
                        lo, hi = qb[qi] * P, qb[qi + 1] * P
                        nc.scalar.activation(
                            out=sscr[:, 0:hi - lo], in_=relu_buf[:, lo:hi],
                            func=AF.Square,
                            accum_out=ss_parts[:, qi:qi + 1])
                        nc.scalar.activation(
                            out=sscr[:, 0:hi - lo], in_=relu_buf[:, lo:hi],
                            func=AF.Copy,
                            accum_out=s_parts[:, qi:qi + 1])
                        qi += 1
                        blackout = BLACKOUT

            # ---- BN statistics
            stats = smpool.tile([P, 2], f32, tag="stats")
            nc.vector.reduce_sum(out=stats[:, 0:1], in_=s_parts[:],
                                 axis=mybir.AxisListType.X)
            nc.vector.reduce_sum(out=stats[:, 1:2], in_=ss_parts[:],
                                 axis=mybir.AxisListType.X)

            # HWDGE (sync) for the SBUF<->DRAM staging legs (~0.6us vs the
            # ~2us SWDGE fixed cost); only the collective itself needs gpsimd
            cc_in = dpool.tile([P, 2], f32, tag="cc_in")
            cc_out = dpool.tile([P, 2], f32, tag="cc_out")
            nc.sync.dma_start(out=cc_in[:], in_=stats[:])
            if with_collective:
                nc.gpsimd.collective_compute(
                    "AllReduce", ALU.add,
                    replica_groups=[list(range(NCORES))],
                    ins=[cc_in.opt()], outs=[cc_out.opt()],
                )
            else:
                nc.gpsimd.dma_start(out=cc_out[:], in_=cc_in[:])
            stats_g = smpool.tile([P, 2], f32, tag="stats_g")
            nc.sync.dma_start(out=stats_g[:], in_=cc_out[:])

            mean = smpool.tile([P, 1], f32, tag="mean")
            nc.vector.tensor_scalar(out=mean[:], in0=stats_g[:, 0:1],
                                    scalar1=1.0 / NV, scalar2=None, op0=ALU.mult)
            ex2 = smpool.tile([P, 1], f32, tag="ex2")
            nc.vector.tensor_scalar(out=ex2[:], in0=stats_g[:, 1:2],
                                    scalar1=1.0 / NV, scalar2=None, op0=ALU.mult)
            msq = smpool.tile([P, 1], f32, tag="msq")
            nc.vector.tensor_tensor(out=msq[:], in0=mean[:], in1=mean[:],
                                    op=ALU.mult)
            var = smpool.tile([P, 1], f32, tag="var")
            nc.vector.tensor_tensor(out=var[:], in0=ex2[:], in1=msq[:],
                                    op=ALU.subtract)
            vare = smpool.tile([P, 1], f32, tag="vare")
            nc.vector.tensor_scalar(out=vare[:], in0=var[:], scalar1=BN_EPS,
                                    scalar2=None, op0=ALU.add)
            std = smpool.tile([P, 1], f32, tag="std")
            nc.scalar.activation(out=std[:], in_=vare[:], func=AF.Sqrt)
            rstd = smpool.tile([P, 1], f32, tag="rstd")
            nc.vector.reciprocal(out=rstd[:], in_=std[:])
            scale = smpool.tile([P, 1], f32, tag="scale")
            nc.vector.tensor_tensor(out=scale[:], in0=gamma_c, in1=rstd[:],
                                    op=ALU.mult)
            nshift = smpool.tile([P, 1], f32, tag="nshift")
            nc.vector.tensor_tensor(out=nshift[:], in0=mean[:], in1=scale[:],
                                    op=ALU.mult)
            shift = smpool.tile([P, 1], f32, tag="shift")
            nc.vector.tensor_tensor(out=shift[:], in0=beta_c, in1=nshift[:],
                                    op=ALU.subtract)

            # ---- fused BN affine + stores, chunked, spread across ACT /
            # DVE / Pool so the chunks run in parallel; stores alternate
            # between the two HWDGE rings.
            aff_eng = [None, nc.vector, None, nc.vector]
            for q in range(NQA):
                lo, hi = qa[q] * P, qa[q + 1] * P
                eng = aff_eng[q % len(aff_eng)]
                if eng is None:
                    nc.scalar.activation(out=out_bf[:, lo:hi],
                                         in_=relu_buf[:, lo:hi],
                                         func=AF.Identity,
                                         scale=scale[:, 0:1],
                                         bias=shift[:, 0:1])
                else:
                    eng.scalar_tensor_tensor(
                        out=out_bf[:, lo:hi], in0=relu_buf[:, lo:hi],
                        scalar=scale[:, 0:1],
                        in1=shift[:, 0:1].to_broadcast([P, hi - lo]),
                        op0=ALU.mult, op1=ALU.add)
                ring = nc.sync if q % 2 == 0 else nc.scalar
                ring.dma_start(out=out_d[:, lo:hi], in_=out_bf[:, lo:hi])

    nc.finalize()
    return nc


# ----------------------------------------------------------------------------
# entry point
# ----------------------------------------------------------------------------

def prepare(inp_dict):
    """Host prep + build device program + per-core input maps (no execution)."""
    import ml_dtypes
    bf16 = ml_dtypes.bfloat16

    inputs = inp_dict["inputs"]
    filt_coeff = inp_dict["filt_coeff"]
    face = inp_dict["face"]
    nf_count = inp_dict["nf_count"]
    vt_map = inp_dict["vt_map"]
    spatial_weights = inp_dict["spatial_weights"]
    depth_weights = inp_dict["depth_weights"]
    biases = inp_dict["biases"]
    gamma = inp_dict["gamma"]
    beta = inp_dict["beta"]

    inpT_cores, filtT_cores, vcore, outcol, T, O, G = _host_prep(
        face, vt_map, nf_count, filt_coeff, inputs)

    # m-major (m,c) layouts so chunk h == m; sw2 replicated at partition
    # bases 0/32/64 to match the filt k-band the PE reads.
    sw2_16 = np.ascontiguousarray(
        np.asarray(spatial_weights, dtype=np.float32)
        .transpose(0, 2, 1).reshape(16, M * C)).astype(bf16)
    FPP = 32 * (KBANDS - 1) + 16
    sw2 = np.zeros((FPP, M * C), dtype=bf16)
    for a in range(KBANDS):
        sw2[32 * a:32 * a + 16] = sw2_16
    dw2 = np.ascontiguousarray(
        np.asarray(depth_weights, dtype=np.float32).reshape(C, M, CO)
        .transpose(1, 0, 2).reshape(M * C, CO)).astype(bf16)

    cp = np.zeros((P, 3), dtype=np.float32)
    cp[:, 0] = np.asarray(biases, dtype=np.float32).reshape(CO)
    cp[:, 1] = np.asarray(gamma, dtype=np.float32).reshape(CO)
    cp[:, 2] = np.asarray(beta, dtype=np.float32).reshape(CO)

    nc = _build_kernel(T, O, G)

    in_maps = []
    for c0 in range(NCORES):
        in_maps.append({
            "inpT": inpT_cores[c0],
            "filtT": filtT_cores[c0],
            "sw2": sw2,
            "dw2": dw2,
            "constpack": cp,
        })

    return {"nc": nc, "in_maps": in_maps, "ncores": NCORES,
            "vcore": vcore, "outcol": outcol, "T": T, "O": O, "G": G}


def kernel(inputs, filt_coeff, face, nf_count, vt_map,
           spatial_weights, depth_weights, biases, gamma, beta):
    from concourse.bass_utils import run_bass_kernel_spmd

    prep = prepare(dict(
        inputs=inputs, filt_coeff=filt_coeff, face=face, nf_count=nf_count,
        vt_map=vt_map, spatial_weights=spatial_weights,
        depth_weights=depth_weights, biases=biases, gamma=gamma, beta=beta))
    nc, in_maps = prep["nc"], prep["in_maps"]
    vcore, outcol = prep["vcore"], prep["outcol"]

    res = run_bass_kernel_spmd(nc, in_maps, core_ids=list(range(NCORES)))
    global _last_results
    _last_results = res
    out = np.zeros((NV, CO), dtype=np.float32)
    percore = [np.asarray(res.results[c]["out_t"], dtype=np.float32)
               for c in range(NCORES)]    # [128o, B*128]
    for c0 in range(NCORES):
        vs = np.where(vcore == c0)[0]
        out[vs] = percore[c0][:, outcol[vs]].T
    return out
